# revision 59
# baseline (speedup 1.0000x reference)
"""KronEmbedding lookup kernel for 8 TRN2 NeuronCores.

Math: w = einsum('sia,sjb->ijab', A, B).reshape(50176, 2048); out = w[x].
Never materializes w. Per token t with i=x//224, j=x%224:
    out[t] = sum_s outer(A[s,i,:], B[s,j,:])   -> (64*32 = 2048 floats)

Strategy (data-parallel over tokens, 1024 tokens/core, all bf16 on the wire):
- Tokens in 64 groups of 16 (k in [0,16)); contraction partition p = 8k+s.
- Per group, two sub-array matmuls (tile_position (0,0)/(64,64)): contraction
  rows 64*hh..64*hh+64 x stationary AG rows -> out partitions (hh, a); moving
  operand bd[64*hh.., g, :] is the token block-diagonal ([256] = 8 tok x 32).
  Both operands contiguous per group (strided rhs is 4x slower on HW).
- bd is 8x zero-padded; shipping it fully costs 11.6us of DMA bus. Hybrid:
    groups < QP:  pre-padded from HBM, loaded FIRST and ALONE on the scalar
                  ring (rings are FIFO; an empty ring drains at full rate),
                  so chunk-0 matmuls start early;
    groups >= QP: zeroed on-device by 8 per-kk-column strip memsets
                  (ACT memzero / DVE / Pool memset) + 16 scatter DMAs of the
                  compact B rows. A scatter waits only on its own strip.
- Queue plan (rings drain queued DMAs round-robin and all HWDGE generations
  serialize on one shared device, so DMA count is budgeted): scalar carries
  the front loads, 4 scatters and out0/1/6; sync carries agr, 6 scatters and
  the remaining outs; gpsimd issues 6 scatters via SWDGE. Strip engines are
  chosen so each ring's scatters wait on strips that finish earliest.
- PSUM evacuation alternates full ps tiles: DVE (h2=0) / ACT (h2=1);
  4 PSUM buffers (warmups rotate through the same pool) hide sem latency.
- Host: upcast bf16 -> fp32 and reorder to token-major (untimed).
"""
import numpy as np
import ml_dtypes
from contextlib import ExitStack

import concourse.bass as bass
import concourse.bacc as bacc
import concourse.tile as tile
import concourse.mybir as mybir
from concourse import bass_utils

dt = mybir.dt
BF16 = ml_dtypes.bfloat16

R, M1, N1, M2, N2 = 8, 224, 64, 224, 32
VOCAB, EMB = M1 * M2, N1 * N2          # 50176, 2048
BATCH, SEQ = 4, 2048
NTOK = BATCH * SEQ                     # 8192
NCORES = 8
TPC = NTOK // NCORES                   # 1024 tokens per core
NGRP = TPC // 16                       # 64 groups of 16 tokens
QP = 24                                # pre-padded leading groups (multiple of 8)
NREST = NGRP - QP                      # scatter-built groups
NWARM = 12

_CACHE = {}


def _build():
    nc = bacc.Bacc("TRN2", num_devices=NCORES)
    AG = nc.dram_tensor("AG", [128, NGRP, 64], dt.bfloat16, kind="ExternalInput")
    if QP:
        BDF = nc.dram_tensor("BDF", [128, QP, 256], dt.bfloat16,
                             kind="ExternalInput")
    GBR = nc.dram_tensor("GBR", [8, 2, 8, NREST, 32], dt.bfloat16,
                         kind="ExternalInput")
    out = nc.dram_tensor("out", [8, 128, 2048], dt.bfloat16, kind="ExternalOutput")

    with tile.TileContext(nc) as tc, ExitStack() as ctx:
        const_pool = ctx.enter_context(tc.tile_pool(name="const", bufs=1))
        ev_pool = ctx.enter_context(tc.tile_pool(name="ev", bufs=10))
        ps_pool = ctx.enter_context(tc.tile_pool(name="ps", bufs=4, space="PSUM"))

        warm = const_pool.tile([128, 512], dt.bfloat16, tag="warm")
        nc.gpsimd.memset(warm[:], 0.0)
        for _ in range(NWARM):
            wps = ps_pool.tile([128, 1024], dt.float32, tag="ps")
            nc.tensor.matmul(wps[:, 0:512], warm[:, 0:128], warm[:],
                             start=True, stop=True)

        # Front (pre-padded) tiles: first and alone on the scalar ring.
        nchunk_f = QP // 8
        bdfs = [const_pool.tile([128, 8, 256], dt.bfloat16, tag=f"bdf{i}",
                                name=f"bdf{i}") for i in range(nchunk_f)]
        agfs = [const_pool.tile([128, 8, 64], dt.bfloat16, tag=f"agf{i}",
                                name=f"agf{i}") for i in range(nchunk_f)]
        agr = const_pool.tile([128, NREST, 64], dt.bfloat16, tag="agr", name="agr")
        bdr = const_pool.tile([128, NREST, 256], dt.bfloat16, tag="bdr", name="bdr")

        for i in range(nchunk_f):
            nc.scalar.dma_start(agfs[i][:], AG[:, 8 * i:8 * i + 8])
            nc.scalar.dma_start(bdfs[i][:], BDF[:, 8 * i:8 * i + 8])

        # Strip memsets of the scatter region, one per kk column block.
        strip_engine = [nc.vector, nc.vector, nc.vector, nc.gpsimd,
                        nc.gpsimd, nc.scalar, nc.scalar, nc.scalar]
        for kk in range(8):
            eng = strip_engine[kk]
            ap = bdr[:, :, 32 * kk:32 * kk + 32]
            if eng is nc.scalar:
                eng.memzero(ap)
            else:
                eng.memset(ap, 0.0)

        # AG for the scattered region goes first on sync (no deps).
        nc.sync.dma_start(agr[:], AG[:, QP:NGRP])
        # Scatters: sync 6, scalar 4 (early, strips done by then), gpsimd 6.
        sc_queue = {0: nc.sync, 1: nc.sync, 2: nc.sync, 3: nc.scalar,
                    4: nc.scalar, 5: nc.gpsimd, 6: nc.gpsimd, 7: nc.gpsimd}
        for kk in range(8):
            for xh in range(2):
                sc_queue[kk].dma_start(
                    bdr[64 * xh + 8 * kk:64 * xh + 8 * kk + 8,
                        :, 32 * kk:32 * kk + 32],
                    GBR[kk, xh],
                )

        # Main stream: 8 chunks x 8 groups; 2 sub-array matmuls per group.
        for chunk in range(8):
            ev = ev_pool.tile([128, 2048], dt.bfloat16, tag="ev")
            for h2 in range(2):
                ps = ps_pool.tile([128, 1024], dt.float32, tag="ps")
                for h in range(4):
                    g = 8 * chunk + 4 * h2 + h
                    if g < QP:
                        agt, agi = agfs[g // 8], g % 8
                        bdt, bgi = bdfs[g // 8], g % 8
                    else:
                        agt, agi = agr, g - QP
                        bdt, bgi = bdr, g - QP
                    for hh in range(2):
                        nc.tensor.matmul(
                            ps[64 * hh:64 * hh + 64, 256 * h:256 * h + 256],
                            agt[64 * hh:64 * hh + 64, agi, :],
                            bdt[64 * hh:64 * hh + 64, bgi, :],
                            start=True,
                            stop=True,
                            tile_position=(64 * hh, 64 * hh),
                        )
                base = 1024 * h2
                if chunk == 7:
                    nc.vector.tensor_copy(ev[:, base:base + 512], ps[:, 0:512])
                    nc.scalar.copy(ev[:, base + 512:base + 1024], ps[:, 512:1024])
                elif h2 == 0:
                    nc.vector.tensor_copy(ev[:, base:base + 1024], ps[:])
                else:
                    nc.scalar.copy(ev[:, base:base + 1024], ps[:])
            if chunk == 7:
                nc.scalar.dma_start(out[7, :, 0:1024], ev[:, 0:1024])
                nc.sync.dma_start(out[7, :, 1024:2048], ev[:, 1024:2048])
            else:
                (nc.scalar if chunk in (0, 1, 6) else nc.sync).dma_start(
                    out[chunk], ev[:])

    nc.compile()
    return nc


def kernel(A: np.ndarray, B: np.ndarray, x: np.ndarray) -> np.ndarray:
    Abf = np.asarray(A, dtype=np.float32).astype(BF16)    # [8, 224, 64]
    Bbf = np.asarray(B, dtype=np.float32).astype(BF16)    # [8, 224, 32]
    xl = np.asarray(x).astype(np.int64).reshape(-1)       # [8192]
    i_all = (xl // M2).astype(np.int64)
    j_all = (xl % M2).astype(np.int64)

    if "nc" not in _CACHE:
        _CACHE["nc"] = _build()
    nc = _CACHE["nc"]

    in_maps = []
    for c in range(NCORES):
        sl = slice(c * TPC, (c + 1) * TPC)
        IA = i_all[sl].reshape(NGRP, 16)                  # [g, k]
        JB = j_all[sl].reshape(NGRP, 16)

        # AG[p, g, a] = A[s, i_t, a], p = 8k+s, t = 16g+k (compact lhsT)
        AGh = np.ascontiguousarray(
            Abf[:, IA, :].transpose(2, 0, 1, 3)           # [16k, 8s, g, a]
        ).reshape(128, NGRP, 64)

        # GB[k, s, g, b] = B[s, j_t, b]
        GB = Bbf[:, JB, :].transpose(2, 0, 1, 3)          # [16k, 8s, g, b]

        im = dict(AG=AGh)
        if QP:
            BDFh = np.zeros((16, 8, QP, 8, 32), dtype=BF16)  # [k, s, g, k8, b]
            for k in range(16):
                BDFh[k, :, :, k % 8, :] = GB[k, :, 0:QP, :]
            im["BDF"] = BDFh.reshape(128, QP, 256)

        # GBR[kk, xh, s, grest, b] = B row of token k = 8*xh + kk.
        im["GBR"] = np.ascontiguousarray(
            GB[:, :, QP:NGRP, :].reshape(2, 8, 8, NREST, 32).transpose(1, 0, 2, 3, 4)
        )
        in_maps.append(im)

    _CACHE["in_maps"] = in_maps
    res = bass_utils.run_bass_kernel_spmd(nc, in_maps, core_ids=list(range(NCORES)))

    outs = []
    for c in range(NCORES):
        o = np.asarray(res.results[c]["out"]).astype(np.float32)  # [8,128,2048]
        # rows: (hh, a); cols within chunk: (h2, h, k8, b), g = 8*chunk+4*h2+h
        o = o.reshape(8, 2, 64, 2, 4, 8, 32)             # [chunk, hh, a, h2, h, k8, b]
        # token t = 16*g + 8*hh + k8 = 128*chunk + 16*(4*h2+h) + 8*hh + k8
        o = o.transpose(0, 3, 4, 1, 5, 2, 6)             # [chunk, h2, h, hh, k8, a, b]
        outs.append(o.reshape(TPC, EMB))
    full = np.concatenate(outs, axis=0)                  # [8192, 2048]
    return full.reshape(BATCH, SEQ, EMB)


# revision 60
# speedup vs baseline: 1.0597x; 1.0597x over previous
"""KronEmbedding lookup kernel for 8 TRN2 NeuronCores.

Math: w = einsum('sia,sjb->ijab', A, B).reshape(50176, 2048); out = w[x].
Never materializes w. Per token t with i=x//224, j=x%224:
    out[t] = sum_s outer(A[s,i,:], B[s,j,:])   -> (64*32 = 2048 floats)

Strategy (data-parallel over tokens, 1024 tokens/core, all bf16 on the wire):
- Tokens in 64 groups of 16 (k in [0,16)); contraction partition p = 8k+s.
- Per group, two sub-array matmuls (tile_position (0,0)/(64,64)): contraction
  rows 64*hh..64*hh+64 x stationary AG rows -> out partitions (hh, a); moving
  operand bd[64*hh.., g, :] is the token block-diagonal ([256] = 8 tok x 32).
  Both operands contiguous per group (strided rhs is 4x slower on HW).
- bd is 8x zero-padded; shipping it fully costs 11.6us of DMA bus. Hybrid:
    groups < QP:  pre-padded from HBM, loaded FIRST and ALONE on the scalar
                  ring (rings are FIFO; an empty ring drains at full rate),
                  so chunk-0 matmuls start early;
    groups >= QP: zeroed on-device by 8 per-kk-column strip memsets
                  (ACT memzero / DVE / Pool memset) + 16 scatter DMAs of the
                  compact B rows. A scatter waits only on its own strip.
- Queue plan (rings drain queued DMAs round-robin and all HWDGE generations
  serialize on one shared device, so DMA count is budgeted): scalar carries
  the front loads, 4 scatters and out0/1/6; sync carries agr, 6 scatters and
  the remaining outs; gpsimd issues 6 scatters via SWDGE. Strip engines are
  chosen so each ring's scatters wait on strips that finish earliest.
- PSUM evacuation alternates full ps tiles: DVE (h2=0) / ACT (h2=1);
  4 PSUM buffers (warmups rotate through the same pool) hide sem latency.
- Host: upcast bf16 -> fp32 and reorder to token-major (untimed).
"""
import numpy as np
import ml_dtypes
from contextlib import ExitStack

import concourse.bass as bass
import concourse.bacc as bacc
import concourse.tile as tile
import concourse.mybir as mybir
from concourse import bass_utils

dt = mybir.dt
BF16 = ml_dtypes.bfloat16

R, M1, N1, M2, N2 = 8, 224, 64, 224, 32
VOCAB, EMB = M1 * M2, N1 * N2          # 50176, 2048
BATCH, SEQ = 4, 2048
NTOK = BATCH * SEQ                     # 8192
NCORES = 8
TPC = NTOK // NCORES                   # 1024 tokens per core
NGRP = TPC // 16                       # 64 groups of 16 tokens
QP = 24                                # pre-padded leading groups (multiple of 8)
NREST = NGRP - QP                      # scatter-built groups
NWARM = 12

_CACHE = {}


def _build():
    nc = bacc.Bacc("TRN2", num_devices=NCORES)
    AG = nc.dram_tensor("AG", [128, NGRP, 64], dt.bfloat16, kind="ExternalInput")
    if QP:
        BDF = nc.dram_tensor("BDF", [128, QP, 256], dt.bfloat16,
                             kind="ExternalInput")
    GBR = nc.dram_tensor("GBR", [8, 2, 8, NREST, 32], dt.bfloat16,
                         kind="ExternalInput")
    out = nc.dram_tensor("out", [8, 128, 2048], dt.bfloat16, kind="ExternalOutput")

    with tile.TileContext(nc) as tc, ExitStack() as ctx:
        const_pool = ctx.enter_context(tc.tile_pool(name="const", bufs=1))
        ev_pool = ctx.enter_context(tc.tile_pool(name="ev", bufs=8))
        ps_pool = ctx.enter_context(tc.tile_pool(name="ps", bufs=4, space="PSUM"))

        warm = const_pool.tile([128, 512], dt.bfloat16, tag="warm")
        nc.gpsimd.memset(warm[:], 0.0)
        for _ in range(NWARM):
            wps = ps_pool.tile([128, 1024], dt.float32, tag="ps")
            nc.tensor.matmul(wps[:, 0:512], warm[:, 0:128], warm[:],
                             start=True, stop=True)

        # Front (pre-padded) tiles: first and alone on the scalar ring.
        nchunk_f = QP // 8
        bdfs = [const_pool.tile([128, 8, 256], dt.bfloat16, tag=f"bdf{i}",
                                name=f"bdf{i}") for i in range(nchunk_f)]
        agfs = [const_pool.tile([128, 8, 64], dt.bfloat16, tag=f"agf{i}",
                                name=f"agf{i}") for i in range(nchunk_f)]
        agr = const_pool.tile([128, NREST, 64], dt.bfloat16, tag="agr", name="agr")
        bdr = const_pool.tile([128, NREST, 256], dt.bfloat16, tag="bdr", name="bdr")

        for i in range(nchunk_f):
            nc.scalar.dma_start(agfs[i][:], AG[:, 8 * i:8 * i + 8])
            nc.scalar.dma_start(bdfs[i][:], BDF[:, 8 * i:8 * i + 8])

        # Strip memsets of the scatter region, one per kk column block.
        strip_engine = [nc.vector, nc.vector, nc.vector, nc.gpsimd,
                        nc.gpsimd, nc.scalar, nc.scalar, nc.scalar]
        for kk in range(8):
            eng = strip_engine[kk]
            ap = bdr[:, :, 32 * kk:32 * kk + 32]
            if eng is nc.scalar:
                eng.memzero(ap)
            else:
                eng.memset(ap, 0.0)

        # AG for the scattered region goes first on sync (no deps).
        nc.sync.dma_start(agr[:], AG[:, QP:NGRP])
        # Scatters: sync 6, scalar 4 (early, strips done by then), gpsimd 6.
        sc_queue = {0: nc.sync, 1: nc.sync, 2: nc.sync, 3: nc.scalar,
                    4: nc.scalar, 5: nc.gpsimd, 6: nc.gpsimd, 7: nc.gpsimd}
        for kk in range(8):
            for xh in range(2):
                sc_queue[kk].dma_start(
                    bdr[64 * xh + 8 * kk:64 * xh + 8 * kk + 8,
                        :, 32 * kk:32 * kk + 32],
                    GBR[kk, xh],
                )

        # Main stream: 8 chunks x 8 groups; 2 sub-array matmuls per group.
        for chunk in range(8):
            ev = ev_pool.tile([128, 2048], dt.bfloat16, tag="ev")
            for h2 in range(2):
                ps = ps_pool.tile([128, 1024], dt.float32, tag="ps")
                for h in range(4):
                    g = 8 * chunk + 4 * h2 + h
                    if g < QP:
                        agt, agi = agfs[g // 8], g % 8
                        bdt, bgi = bdfs[g // 8], g % 8
                    else:
                        agt, agi = agr, g - QP
                        bdt, bgi = bdr, g - QP
                    for hh in range(2):
                        nc.tensor.matmul(
                            ps[64 * hh:64 * hh + 64, 256 * h:256 * h + 256],
                            agt[64 * hh:64 * hh + 64, agi, :],
                            bdt[64 * hh:64 * hh + 64, bgi, :],
                            start=True,
                            stop=True,
                            tile_position=(64 * hh, 64 * hh),
                        )
                base = 1024 * h2
                if chunk == 7:
                    nc.vector.tensor_copy(ev[:, base:base + 512], ps[:, 0:512])
                    nc.scalar.copy(ev[:, base + 512:base + 1024], ps[:, 512:1024])
                elif h2 == 0:
                    nc.vector.tensor_copy(ev[:, base:base + 1024], ps[:])
                else:
                    nc.scalar.copy(ev[:, base:base + 1024], ps[:])
            if chunk == 7:
                nc.scalar.dma_start(out[7, :, 0:1024], ev[:, 0:1024])
                nc.sync.dma_start(out[7, :, 1024:2048], ev[:, 1024:2048])
            else:
                (nc.scalar if chunk in (0, 1, 6) else nc.sync).dma_start(
                    out[chunk], ev[:])

    nc.compile()
    return nc


def kernel(A: np.ndarray, B: np.ndarray, x: np.ndarray) -> np.ndarray:
    Abf = np.asarray(A, dtype=np.float32).astype(BF16)    # [8, 224, 64]
    Bbf = np.asarray(B, dtype=np.float32).astype(BF16)    # [8, 224, 32]
    xl = np.asarray(x).astype(np.int64).reshape(-1)       # [8192]
    i_all = (xl // M2).astype(np.int64)
    j_all = (xl % M2).astype(np.int64)

    if "nc" not in _CACHE:
        _CACHE["nc"] = _build()
    nc = _CACHE["nc"]

    in_maps = []
    for c in range(NCORES):
        sl = slice(c * TPC, (c + 1) * TPC)
        IA = i_all[sl].reshape(NGRP, 16)                  # [g, k]
        JB = j_all[sl].reshape(NGRP, 16)

        # AG[p, g, a] = A[s, i_t, a], p = 8k+s, t = 16g+k (compact lhsT)
        AGh = np.ascontiguousarray(
            Abf[:, IA, :].transpose(2, 0, 1, 3)           # [16k, 8s, g, a]
        ).reshape(128, NGRP, 64)

        # GB[k, s, g, b] = B[s, j_t, b]
        GB = Bbf[:, JB, :].transpose(2, 0, 1, 3)          # [16k, 8s, g, b]

        im = dict(AG=AGh)
        if QP:
            BDFh = np.zeros((16, 8, QP, 8, 32), dtype=BF16)  # [k, s, g, k8, b]
            for k in range(16):
                BDFh[k, :, :, k % 8, :] = GB[k, :, 0:QP, :]
            im["BDF"] = BDFh.reshape(128, QP, 256)

        # GBR[kk, xh, s, grest, b] = B row of token k = 8*xh + kk.
        im["GBR"] = np.ascontiguousarray(
            GB[:, :, QP:NGRP, :].reshape(2, 8, 8, NREST, 32).transpose(1, 0, 2, 3, 4)
        )
        in_maps.append(im)

    _CACHE["in_maps"] = in_maps
    res = bass_utils.run_bass_kernel_spmd(nc, in_maps, core_ids=list(range(NCORES)))

    outs = []
    for c in range(NCORES):
        o = np.asarray(res.results[c]["out"]).astype(np.float32)  # [8,128,2048]
        # rows: (hh, a); cols within chunk: (h2, h, k8, b), g = 8*chunk+4*h2+h
        o = o.reshape(8, 2, 64, 2, 4, 8, 32)             # [chunk, hh, a, h2, h, k8, b]
        # token t = 16*g + 8*hh + k8 = 128*chunk + 16*(4*h2+h) + 8*hh + k8
        o = o.transpose(0, 3, 4, 1, 5, 2, 6)             # [chunk, h2, h, hh, k8, a, b]
        outs.append(o.reshape(TPC, EMB))
    full = np.concatenate(outs, axis=0)                  # [8192, 2048]
    return full.reshape(BATCH, SEQ, EMB)
